# revision 7
# baseline (speedup 1.0000x reference)
"""Expert-parallel MoE MLP (BaseMLPExperts) for 8 TRN2 NeuronCores.

Reference computation (per expert e):
    y[:, e, :] = gelu_exact(x[:, e, :] @ wi[e]) @ wo[e]
with T=8192 tokens, E=8 experts, H=1024 hidden, I=4096 intermediate, fp32.

Sharding: expert-parallel — core e owns expert e (its x slice, wi[e], wo[e]).
No cross-core communication.

Per-core device kernel, v3: all matmul operands in bf16 (fp32 PSUM
accumulation; end-to-end rel-err ~3e-3, inside the 2e-2 gate). bf16
halves every DMA stream vs f32r and enables the PE fast-weight-load
path, so the 512-row matmul cadence sits at the issue floor (~216ns).

  Phase 1: h1T[I, T] = gelu(x @ wi) streamed by 512-token tiles; wi
           fully SBUF-resident in bf16 (64KB/partition), streamed in
           256-column pieces round-robin over the three DMA queues in
           consumption order; GELU applied on PSUM eviction by ACT,
           written to DRAM scratch as bf16. Token tiles 0/1 run
           interleaved i-major so wi streaming stays ahead; wo
           (64KB/partition) prefetches behind it. A dozen dummy matmuls
           on a memset scratch tile warm the PE clock (HAM un-throttle)
           while the priming DMA is still in flight.
  Phase 2: y[T, H] = h1 @ wo by 128-token blocks; h1T tiles are the
           stationary operand so y comes out untransposed; h1 loads
           fetch two blocks per DMA (512B/partition lines). One PSUM
           pool is shared across phases so there is no transition
           barrier; the last block runs h-half-major so its first
           y-half stores while the second half is still accumulating.
"""

import numpy as np

import concourse.bass as bass
import concourse.mybir as mybir
import concourse.tile as tile
from concourse import bacc
from concourse.bass_utils import run_bass_kernel_spmd

T, E, H, I = 8192, 8, 1024, 4096
P = 128
F32 = mybir.dt.float32
BF16 = mybir.dt.bfloat16

TT1 = 512            # phase-1 token tile
NT1 = T // TT1       # 16
HT = H // P          # 8 k-tiles for GEMM1
IT = I // P          # 32 i-tiles
TT2 = 128            # phase-2 token block
NT2 = T // TT2       # 64

# run_bass_kernel_spmd kwargs injected by test harness (e.g. trace=True)
RUN_KWARGS: dict = {}
LAST_RESULT = None

_NC = None


def _build():
    nc = bacc.Bacc("TRN2", target_bir_lowering=False, debug=False, num_devices=8)

    xT = nc.dram_tensor("xT", [H, T], BF16, kind="ExternalInput").ap()
    wi = nc.dram_tensor("wi", [H, I], BF16, kind="ExternalInput").ap()
    wo = nc.dram_tensor("wo", [I, H], BF16, kind="ExternalInput").ap()
    y = nc.dram_tensor("y", [T, H], F32, kind="ExternalOutput").ap()

    xT_r = xT.rearrange("(ho p) t -> p ho t", p=P)      # [128, 8, T]
    wi_r = wi.rearrange("(ho p) i -> p ho i", p=P)      # [128, 8, I]
    wo_r = wo.rearrange("(io p) h -> p io h", p=P)      # [128, 32, H]

    with tile.TileContext(nc) as tc:
        with tc.tile_pool(name="h1dram", bufs=1, space="DRAM") as dpool:
            # h1T scratch: one [I, TT1] block per phase-1 token tile
            h1b = [
                dpool.tile([I, TT1], BF16, name=f"h1b{t}", tag=f"h1b{t}")
                for t in range(NT1)
            ]

            w_pool = tc.alloc_tile_pool(name="w_pool", bufs=1)
            wi_sb = w_pool.tile([P, HT, I], BF16, name="wi_sb")
            wo_sb = w_pool.tile([P, IT, H], BF16, name="wo_sb")

            # phase-2 h1 input pool hoisted so its first loads can be
            # issued from inside phase 1 (prefetch under compute)
            h1i_pool = tc.alloc_tile_pool(name="h1i_pool", bufs=2)

            def load_h1i(pair, engs=None):
                # h1 for token blocks 2*pair, 2*pair+1 (256 tokens)
                tt, tsub = pair // 2, pair % 2
                src = h1b[tt].rearrange("(io p) t -> p io t", p=P)
                h1i = h1i_pool.tile([P, IT, 2 * TT2], BF16, name="h1i", tag="h1i")
                if engs is None:
                    engs = (nc.sync, nc.gpsimd)
                for g, eng in enumerate(engs):
                    eng.dma_start(
                        out=h1i[:, 16 * g : 16 * g + 16, :],
                        in_=src[
                            :,
                            16 * g : 16 * g + 16,
                            tsub * 2 * TT2 : (tsub + 1) * 2 * TT2,
                        ],
                    )
                return h1i

            with (
                tc.tile_pool(name="xt_pool", bufs=3) as xt_pool,
                tc.tile_pool(name="h1o_pool", bufs=6) as h1o_pool,
                tc.tile_pool(name="ps_pool", bufs=8, space="PSUM") as ps_pool,
            ):
                def load_xt(tt, split=2):
                    t0 = tt * TT1
                    xt = xt_pool.tile([P, HT, TT1], BF16, name="xt", tag="xt")
                    engs = [nc.sync, nc.scalar, nc.sync, nc.scalar]
                    step = HT // split
                    for q in range(split):
                        engs[q].dma_start(
                            out=xt[:, step * q : step * (q + 1), :],
                            in_=xT_r[:, step * q : step * (q + 1), t0 : t0 + TT1],
                        )
                    return xt

                # PE warmup: a dozen dummy matmuls on a memset scratch
                # tile run while priming DMA is in flight, so the HAM
                # clock gate opens (1.2 -> 2.4 GHz) before real work.
                warm = h1o_pool.tile([P, TT1], BF16, name="h1o", tag="h1o")
                nc.vector.memset(warm[:], 0.0)
                for _ in range(12):
                    wps = ps_pool.tile([P, TT1], F32, name="wps", tag="ps")
                    nc.tensor.matmul(
                        wps[:], warm[:, 0:P], warm[:], start=True, stop=True
                    )

                # wi streaming, in consumption order. DMA efficiency is
                # set by the per-partition contiguous line length, so
                # beyond the head the pieces are [one ho row x many
                # i-columns] (1.5-2KB lines) instead of column slabs
                # (512B lines).
                def load_wi(h0, h1, c0, c1, eng):
                    eng.dma_start(
                        out=wi_sb[:, h0:h1, c0:c1],
                        in_=wi_r[:, h0:h1, c0:c1],
                    )

                xt0 = load_xt(0, split=4)
                # head: igroups 0-1 (all ho, 512B lines, GpSimd SWDGE)
                load_wi(0, HT, 0, 128, nc.gpsimd)
                load_wi(0, HT, 128, 256, nc.gpsimd)
                xt1 = load_xt(1, split=4)
                engs3 = [nc.sync, nc.scalar, nc.gpsimd]
                # q1 rows 0-3 ride GpSimd before the h1 stores queue up
                for ho in range(4):
                    load_wi(ho, ho + 1, 1024, 2048, nc.gpsimd)
                # q0 remainder in column slabs (fine-grained unblocking)
                for j, eng in ((0, nc.sync), (1, nc.scalar), (2, nc.sync)):
                    load_wi(0, HT, 256 * (j + 1), 256 * (j + 2), eng)
                # q0 col 768:1024, q1 rows 4-7, then q2/q3 row pieces
                load_wi(0, HT, 768, 1024, nc.scalar)
                for ho in range(4, HT):
                    load_wi(ho, ho + 1, 1024, 2048, engs3[ho % 2])
                for q in (2, 3):
                    for ho in range(HT):
                        load_wi(ho, ho + 1, q * 1024, (q + 1) * 1024, engs3[ho % 2])

                def igroup(tt, i, xt):
                    ps = ps_pool.tile([P, TT1], F32, name="ps1", tag="ps")
                    for h in range(HT):
                        nc.tensor.matmul(
                            ps[:],
                            wi_sb[:, h, i * P : (i + 1) * P],
                            xt[:, h, :],
                            start=(h == 0),
                            stop=(h == HT - 1),
                        )
                    h1o = h1o_pool.tile([P, TT1], BF16, name="h1o", tag="h1o")
                    nc.scalar.activation(
                        h1o[:], ps[:], mybir.ActivationFunctionType.Gelu
                    )
                    nc.gpsimd.dma_start(
                        out=h1b[tt][i * P : (i + 1) * P, :], in_=h1o[:]
                    )

                # Token tiles 0 and 1 interleaved i-major: halves the wi
                # consumption rate while the priming burst streams in.
                for i in range(IT):
                    igroup(0, i, xt0)
                    igroup(1, i, xt1)
                    if i == 8:
                        xt_cur = load_xt(2)

                for tt in range(2, NT1):
                    xt_nxt = load_xt(tt + 1) if tt + 1 < NT1 else None
                    for i in range(IT):
                        igroup(tt, i, xt_cur)
                    if tt in (2, 3):
                        # wo prefetch under phase-1 compute: 4 pieces of
                        # 4 i-tiles (1MB each) per tile on SP/ACT
                        for g in range(4 * (tt - 2), 4 * (tt - 1)):
                            engs3[g % 2].dma_start(
                                out=wo_sb[:, 4 * g : 4 * g + 4, :],
                                in_=wo_r[:, 4 * g : 4 * g + 4, :],
                            )
                    if tt == 10:
                        pending = [load_h1i(0, (nc.sync, nc.scalar))]
                    if tt == 13:
                        pending.append(load_h1i(1, (nc.sync, nc.scalar)))
                    xt_cur = xt_nxt

                # ---------------- Phase 2: y = h1 @ wo ------------------
                with tc.tile_pool(name="yo_pool", bufs=3) as yo_pool:
                    for tb in range(NT2):
                        if tb % 2 == 0:
                            h1i = pending.pop(0)
                            if tb // 2 + 2 < NT2 // 2:
                                pending.append(load_h1i(tb // 2 + 2))
                        tcol = (tb % 2) * TT2
                        yo = yo_pool.tile([P, H], F32, name="yo", tag="yo")
                        if tb < NT2 - 1:
                            # i outer / h-half inner: each stationary h1
                            # tile feeds two matmuls back to back
                            pss = [
                                ps_pool.tile([P, 512], F32, name="ps2", tag="ps")
                                for _ in range(2)
                            ]
                            for i in range(IT):
                                for hh in range(2):
                                    nc.tensor.matmul(
                                        pss[hh][:],
                                        h1i[:, i, tcol : tcol + TT2],
                                        wo_sb[:, i, hh * 512 : (hh + 1) * 512],
                                        start=(i == 0),
                                        stop=(i == IT - 1),
                                    )
                            for hh in range(2):
                                nc.vector.tensor_copy(
                                    yo[:, hh * 512 : (hh + 1) * 512], pss[hh][:]
                                )
                            nc.scalar.dma_start(
                                out=y[tb * TT2 : (tb + 1) * TT2, :], in_=yo[:]
                            )
                        else:
                            # last block h-half-major: store the first
                            # y-half while the second half accumulates
                            for hh in range(2):
                                ps = ps_pool.tile([P, 512], F32, name="ps2", tag="ps")
                                for i in range(IT):
                                    nc.tensor.matmul(
                                        ps[:],
                                        h1i[:, i, tcol : tcol + TT2],
                                        wo_sb[:, i, hh * 512 : (hh + 1) * 512],
                                        start=(i == 0),
                                        stop=(i == IT - 1),
                                    )
                                nc.vector.tensor_copy(
                                    yo[:, hh * 512 : (hh + 1) * 512], ps[:]
                                )
                                nc.scalar.dma_start(
                                    out=y[
                                        tb * TT2 : (tb + 1) * TT2,
                                        hh * 512 : (hh + 1) * 512,
                                    ],
                                    in_=yo[:, hh * 512 : (hh + 1) * 512],
                                )
            h1i_pool.release()
            w_pool.release()

    nc.compile()
    return nc


def kernel(x: np.ndarray, wi: np.ndarray, wo: np.ndarray) -> np.ndarray:
    global _NC, LAST_RESULT
    import ml_dtypes

    bf = ml_dtypes.bfloat16
    x = np.asarray(x, dtype=np.float32)
    wi = np.asarray(wi, dtype=np.float32)
    wo = np.asarray(wo, dtype=np.float32)
    assert x.shape == (T, E, H) and wi.shape == (E, H, I) and wo.shape == (E, I, H)

    if _NC is None:
        _NC = _build()

    in_maps = [
        {
            "xT": np.ascontiguousarray(x[:, e, :].T.astype(bf)),
            "wi": np.ascontiguousarray(wi[e].astype(bf)),
            "wo": np.ascontiguousarray(wo[e].astype(bf)),
        }
        for e in range(E)
    ]
    try:
        res = run_bass_kernel_spmd(
            _NC, in_maps, core_ids=list(range(E)), **RUN_KWARGS
        )
    except Exception:
        res = run_bass_kernel_spmd(
            _NC, in_maps, core_ids=list(range(E)), **RUN_KWARGS
        )
    LAST_RESULT = res
    out = np.stack([res.results[e]["y"] for e in range(E)], axis=1)
    return np.ascontiguousarray(out.astype(np.float32, copy=False))


# revision 10
# speedup vs baseline: 1.0060x; 1.0060x over previous
"""Expert-parallel MoE MLP (BaseMLPExperts) for 8 TRN2 NeuronCores.

Reference computation (per expert e):
    y[:, e, :] = gelu_exact(x[:, e, :] @ wi[e]) @ wo[e]
with T=8192 tokens, E=8 experts, H=1024 hidden, I=4096 intermediate, fp32.

Sharding: expert-parallel — core e owns expert e (its x slice, wi[e], wo[e]).
No cross-core communication.

Per-core device kernel, v3: all matmul operands in bf16 (fp32 PSUM
accumulation; end-to-end rel-err ~3e-3, inside the 2e-2 gate). bf16
halves every DMA stream vs f32r and enables the PE fast-weight-load
path, so the 512-row matmul cadence sits at the issue floor (~216ns).

  Phase 1: h1T[I, T] = gelu(x @ wi) streamed by 512-token tiles; wi
           fully SBUF-resident in bf16 (64KB/partition), streamed in
           256-column pieces round-robin over the three DMA queues in
           consumption order; GELU applied on PSUM eviction by ACT,
           written to DRAM scratch as bf16. Token tiles 0/1 run
           interleaved i-major so wi streaming stays ahead; wo
           (64KB/partition) prefetches behind it. A dozen dummy matmuls
           on a memset scratch tile warm the PE clock (HAM un-throttle)
           while the priming DMA is still in flight.
  Phase 2: y[T, H] = h1 @ wo by 128-token blocks; h1T tiles are the
           stationary operand so y comes out untransposed; h1 loads
           fetch two blocks per DMA (512B/partition lines). One PSUM
           pool is shared across phases so there is no transition
           barrier; the last block runs h-half-major so its first
           y-half stores while the second half is still accumulating.
"""

import numpy as np

import concourse.bass as bass
import concourse.mybir as mybir
import concourse.tile as tile
from concourse import bacc
from concourse.bass_utils import run_bass_kernel_spmd

T, E, H, I = 8192, 8, 1024, 4096
P = 128
F32 = mybir.dt.float32
BF16 = mybir.dt.bfloat16

TT1 = 512            # phase-1 token tile
NT1 = T // TT1       # 16
HT = H // P          # 8 k-tiles for GEMM1
IT = I // P          # 32 i-tiles
TT2 = 128            # phase-2 token block
NT2 = T // TT2       # 64

# run_bass_kernel_spmd kwargs injected by test harness (e.g. trace=True)
RUN_KWARGS: dict = {}
LAST_RESULT = None

_NC = None


def _build():
    nc = bacc.Bacc("TRN2", target_bir_lowering=False, debug=False, num_devices=8)

    xT = nc.dram_tensor("xT", [H, T], BF16, kind="ExternalInput").ap()
    wi = nc.dram_tensor("wi", [H, I], BF16, kind="ExternalInput").ap()
    wo = nc.dram_tensor("wo", [I, H], BF16, kind="ExternalInput").ap()
    y = nc.dram_tensor("y", [T, H], F32, kind="ExternalOutput").ap()

    xT_r = xT.rearrange("(ho p) t -> p ho t", p=P)      # [128, 8, T]
    wi_r = wi.rearrange("(ho p) i -> p ho i", p=P)      # [128, 8, I]
    wo_r = wo.rearrange("(io p) h -> p io h", p=P)      # [128, 32, H]

    with tile.TileContext(nc) as tc:
        with tc.tile_pool(name="h1dram", bufs=1, space="DRAM") as dpool:
            # h1T scratch: one [I, TT1] block per phase-1 token tile
            h1b = [
                dpool.tile([I, TT1], BF16, name=f"h1b{t}", tag=f"h1b{t}")
                for t in range(NT1)
            ]

            w_pool = tc.alloc_tile_pool(name="w_pool", bufs=1)
            wi_sb = w_pool.tile([P, HT, I], BF16, name="wi_sb")
            wo_sb = w_pool.tile([P, IT, H], BF16, name="wo_sb")

            # phase-2 h1 input pool hoisted so its first loads can be
            # issued from inside phase 1 (prefetch under compute)
            h1i_pool = tc.alloc_tile_pool(name="h1i_pool", bufs=2)

            def load_h1i(pair, engs=None):
                # h1 for token blocks 2*pair, 2*pair+1 (256 tokens)
                tt, tsub = pair // 2, pair % 2
                src = h1b[tt].rearrange("(io p) t -> p io t", p=P)
                h1i = h1i_pool.tile([P, IT, 2 * TT2], BF16, name="h1i", tag="h1i")
                if engs is None:
                    engs = (nc.sync, nc.gpsimd)
                for g, eng in enumerate(engs):
                    eng.dma_start(
                        out=h1i[:, 16 * g : 16 * g + 16, :],
                        in_=src[
                            :,
                            16 * g : 16 * g + 16,
                            tsub * 2 * TT2 : (tsub + 1) * 2 * TT2,
                        ],
                    )
                return h1i

            with (
                tc.tile_pool(name="xt_pool", bufs=3) as xt_pool,
                tc.tile_pool(name="h1o_pool", bufs=8) as h1o_pool,
                tc.tile_pool(name="ps_pool", bufs=8, space="PSUM") as ps_pool,
            ):
                def load_xt(tt, split=2):
                    t0 = tt * TT1
                    xt = xt_pool.tile([P, HT, TT1], BF16, name="xt", tag="xt")
                    engs = [nc.sync, nc.scalar, nc.sync, nc.scalar]
                    step = HT // split
                    for q in range(split):
                        engs[q].dma_start(
                            out=xt[:, step * q : step * (q + 1), :],
                            in_=xT_r[:, step * q : step * (q + 1), t0 : t0 + TT1],
                        )
                    return xt

                # PE warmup: a dozen dummy matmuls on a memset scratch
                # tile run while priming DMA is in flight, so the HAM
                # clock gate opens (1.2 -> 2.4 GHz) before real work.
                warm = h1o_pool.tile([P, TT1], BF16, name="h1o", tag="h1o")
                nc.vector.memset(warm[:], 0.0)
                for _ in range(12):
                    wps = ps_pool.tile([P, TT1], F32, name="wps", tag="ps")
                    nc.tensor.matmul(
                        wps[:], warm[:, 0:P], warm[:], start=True, stop=True
                    )

                # wi streaming, in consumption order. DMA efficiency is
                # set by the per-partition contiguous line length, so
                # beyond the head the pieces are [one ho row x many
                # i-columns] (1.5-2KB lines) instead of column slabs
                # (512B lines).
                def load_wi(h0, h1, c0, c1, eng):
                    eng.dma_start(
                        out=wi_sb[:, h0:h1, c0:c1],
                        in_=wi_r[:, h0:h1, c0:c1],
                    )

                xt0 = load_xt(0, split=4)
                # head: igroups 0-1 (all ho, 512B lines, GpSimd SWDGE)
                load_wi(0, HT, 0, 128, nc.gpsimd)
                load_wi(0, HT, 128, 256, nc.gpsimd)
                xt1 = load_xt(1, split=4)
                # Hoist the ~1.3us GELU activation-table load (and keep
                # the in-order ACT queue clear of further DMA configs)
                # so the first real PSUM eviction isn't stuck behind it:
                # a dummy SBUF gelu right after the xt configs forces the
                # table load during priming.
                dummy_gelu = h1o_pool.tile([P, TT1], BF16, name="h1o", tag="h1o")
                nc.scalar.activation(
                    dummy_gelu[:], warm[:], mybir.ActivationFunctionType.Gelu
                )
                # q1 rows 0-3 ride GpSimd before the h1 stores queue up;
                # everything else streams on SP (ACT stays gelu-only).
                for ho in range(4):
                    load_wi(ho, ho + 1, 1024, 2048, nc.gpsimd)
                # q0 remainder in column slabs (fine-grained unblocking)
                for j in range(3):
                    load_wi(0, HT, 256 * (j + 1), 256 * (j + 2), nc.sync)
                for ho in range(4, HT):
                    load_wi(ho, ho + 1, 1024, 2048, nc.sync)
                for q in (2, 3):
                    for ho in range(HT):
                        load_wi(ho, ho + 1, q * 1024, (q + 1) * 1024, nc.sync)

                def igroup(tt, i, xt):
                    ps = ps_pool.tile([P, TT1], F32, name="ps1", tag="ps")
                    for h in range(HT):
                        nc.tensor.matmul(
                            ps[:],
                            wi_sb[:, h, i * P : (i + 1) * P],
                            xt[:, h, :],
                            start=(h == 0),
                            stop=(h == HT - 1),
                        )
                    h1o = h1o_pool.tile([P, TT1], BF16, name="h1o", tag="h1o")
                    nc.scalar.activation(
                        h1o[:], ps[:], mybir.ActivationFunctionType.Gelu
                    )
                    nc.gpsimd.dma_start(
                        out=h1b[tt][i * P : (i + 1) * P, :], in_=h1o[:]
                    )

                # Token tiles 0 and 1 interleaved i-major: halves the wi
                # consumption rate while the priming burst streams in.
                for i in range(IT):
                    igroup(0, i, xt0)
                    igroup(1, i, xt1)
                    if i == 8:
                        xt_cur = load_xt(2)

                for tt in range(2, NT1):
                    xt_nxt = load_xt(tt + 1) if tt + 1 < NT1 else None
                    for i in range(IT):
                        igroup(tt, i, xt_cur)
                    if tt in (2, 3):
                        # wo prefetch under phase-1 compute on SP (ACT
                        # stays gelu-only): 4 pieces of 4 i-tiles per tt
                        for g in range(4 * (tt - 2), 4 * (tt - 1)):
                            nc.sync.dma_start(
                                out=wo_sb[:, 4 * g : 4 * g + 4, :],
                                in_=wo_r[:, 4 * g : 4 * g + 4, :],
                            )
                    if tt == 10:
                        pending = [load_h1i(0, (nc.sync, nc.sync))]
                    if tt == 13:
                        pending.append(load_h1i(1, (nc.sync, nc.sync)))
                    xt_cur = xt_nxt

                # ---------------- Phase 2: y = h1 @ wo ------------------
                with tc.tile_pool(name="yo_pool", bufs=3) as yo_pool:
                    for tb in range(NT2):
                        if tb % 2 == 0:
                            h1i = pending.pop(0)
                            if tb // 2 + 2 < NT2 // 2:
                                pending.append(load_h1i(tb // 2 + 2))
                        tcol = (tb % 2) * TT2
                        yo = yo_pool.tile([P, H], F32, name="yo", tag="yo")
                        if tb < NT2 - 1:
                            # i outer / h-half inner: each stationary h1
                            # tile feeds two matmuls back to back
                            pss = [
                                ps_pool.tile([P, 512], F32, name="ps2", tag="ps")
                                for _ in range(2)
                            ]
                            for i in range(IT):
                                for hh in range(2):
                                    nc.tensor.matmul(
                                        pss[hh][:],
                                        h1i[:, i, tcol : tcol + TT2],
                                        wo_sb[:, i, hh * 512 : (hh + 1) * 512],
                                        start=(i == 0),
                                        stop=(i == IT - 1),
                                    )
                            for hh in range(2):
                                nc.vector.tensor_copy(
                                    yo[:, hh * 512 : (hh + 1) * 512], pss[hh][:]
                                )
                            nc.scalar.dma_start(
                                out=y[tb * TT2 : (tb + 1) * TT2, :], in_=yo[:]
                            )
                        else:
                            # last block h-half-major: store the first
                            # y-half while the second half accumulates
                            for hh in range(2):
                                ps = ps_pool.tile([P, 512], F32, name="ps2", tag="ps")
                                for i in range(IT):
                                    nc.tensor.matmul(
                                        ps[:],
                                        h1i[:, i, tcol : tcol + TT2],
                                        wo_sb[:, i, hh * 512 : (hh + 1) * 512],
                                        start=(i == 0),
                                        stop=(i == IT - 1),
                                    )
                                nc.vector.tensor_copy(
                                    yo[:, hh * 512 : (hh + 1) * 512], ps[:]
                                )
                                nc.scalar.dma_start(
                                    out=y[
                                        tb * TT2 : (tb + 1) * TT2,
                                        hh * 512 : (hh + 1) * 512,
                                    ],
                                    in_=yo[:, hh * 512 : (hh + 1) * 512],
                                )
            h1i_pool.release()
            w_pool.release()

    nc.compile()
    return nc


def kernel(x: np.ndarray, wi: np.ndarray, wo: np.ndarray) -> np.ndarray:
    global _NC, LAST_RESULT
    import ml_dtypes

    bf = ml_dtypes.bfloat16
    x = np.asarray(x, dtype=np.float32)
    wi = np.asarray(wi, dtype=np.float32)
    wo = np.asarray(wo, dtype=np.float32)
    assert x.shape == (T, E, H) and wi.shape == (E, H, I) and wo.shape == (E, I, H)

    if _NC is None:
        _NC = _build()

    in_maps = [
        {
            "xT": np.ascontiguousarray(x[:, e, :].T.astype(bf)),
            "wi": np.ascontiguousarray(wi[e].astype(bf)),
            "wo": np.ascontiguousarray(wo[e].astype(bf)),
        }
        for e in range(E)
    ]
    try:
        res = run_bass_kernel_spmd(
            _NC, in_maps, core_ids=list(range(E)), **RUN_KWARGS
        )
    except Exception:
        res = run_bass_kernel_spmd(
            _NC, in_maps, core_ids=list(range(E)), **RUN_KWARGS
        )
    LAST_RESULT = res
    out = np.stack([res.results[e]["y"] for e in range(E)], axis=1)
    return np.ascontiguousarray(out.astype(np.float32, copy=False))
